# revision 1
# baseline (speedup 1.0000x reference)
"""Bass kernel builder for GatedDeltaNet — 8-core SPMD, head-parallel.

Per core: 4 v-heads, 2 k-heads. Layout: projections produce [feature, token]
(feature on partitions); delta-rule chunk math at C=128 with token-on-partition
tiles produced by PE transposes; truncated Neumann-product triangular solve
(3 factors, host-validated to 1.8e-6).
"""
import numpy as np
import concourse.bass as bass
import concourse.mybir as mybir
import concourse.tile as tile
from concourse import bacc
from contextlib import ExitStack

F32 = mybir.dt.float32
BF16 = mybir.dt.bfloat16
AL = mybir.AluOpType
AF = mybir.ActivationFunctionType

HID = 2048
C = 128          # chunk length
BT = 512         # token block
NBLK = 16        # 2 batches x 8 blocks
NCH = 4          # chunks per block
KT = 16          # hidden k-tiles
EPS = 1e-6
STAGE = 4


def build_nc():
    nc = bacc.Bacc("TRN2", target_bir_lowering=False, debug=False, num_devices=8)
    d = {}
    d["x"] = nc.dram_tensor("x", (8192, HID), F32, kind="ExternalInput")
    d["wcat"] = nc.dram_tensor("wcat", (128, KT * 1536), BF16, kind="ExternalInput")
    d["wab"] = nc.dram_tensor("wab", (128, KT * 8), BF16, kind="ExternalInput")
    d["wout"] = nc.dram_tensor("wout", (128, 4 * 2048), BF16, kind="ExternalInput")
    d["convw"] = nc.dram_tensor("convw", (128, 32), F32, kind="ExternalInput")
    d["dtbnea"] = nc.dram_tensor("dtbnea", (128, 8), F32, kind="ExternalInput")
    d["idf32"] = nc.dram_tensor("idf32", (128, 128), F32, kind="ExternalInput")
    d["idbf"] = nc.dram_tensor("idbf", (128, 128), BF16, kind="ExternalInput")
    d["penUi"] = nc.dram_tensor("penUi", (128, 512), F32, kind="ExternalInput")
    d["penUs"] = nc.dram_tensor("penUs", (128, 512), F32, kind="ExternalInput")
    d["penLs"] = nc.dram_tensor("penLs", (128, 512), F32, kind="ExternalInput")
    d["uincl"] = nc.dram_tensor("uincl", (128, 128), F32, kind="ExternalInput")
    d["lstrict"] = nc.dram_tensor("lstrict", (128, 128), F32, kind="ExternalInput")
    d["onesbf"] = nc.dram_tensor("onesbf", (128, 1), BF16, kind="ExternalInput")
    d["onesrow"] = nc.dram_tensor("onesrow", (1, 128), BF16, kind="ExternalInput")
    d["qscale"] = nc.dram_tensor("qscale", (1, 128), BF16, kind="ExternalInput")
    d["onesf32"] = nc.dram_tensor("onesf32", (1, 128), F32, kind="ExternalInput")
    d["ones128"] = nc.dram_tensor("ones128", (128, 128), F32, kind="ExternalInput")
    d["epscol"] = nc.dram_tensor("epscol", (128, 1), F32, kind="ExternalInput")
    d["y"] = nc.dram_tensor("y", (8192, HID), F32, kind="ExternalOutput")

    with tile.TileContext(nc) as tc, ExitStack() as ctx:
        cp = ctx.enter_context(tc.tile_pool(name="consts", bufs=1))
        # resident weights/constants
        wcat = cp.tile([128, KT * 1536], BF16, tag="wcat")
        wab = cp.tile([128, KT * 8], BF16, tag="wab")
        wout = cp.tile([128, 4 * 2048], BF16, tag="wout")
        convw = cp.tile([128, 32], F32, tag="convw")
        dtbnea = cp.tile([128, 8], F32, tag="dtbnea")
        idf32 = cp.tile([128, 128], F32, tag="idf32")
        idbf = cp.tile([128, 128], BF16, tag="idbf")
        penUi = cp.tile([128, 512], F32, tag="penUi")
        penUs = cp.tile([128, 512], F32, tag="penUs")
        penLs = cp.tile([128, 512], F32, tag="penLs")
        uincl = cp.tile([128, 128], F32, tag="uincl")
        lstrict = cp.tile([128, 128], F32, tag="lstrict")
        onesbf = cp.tile([128, 1], BF16, tag="onesbf")
        onesrow = cp.tile([1, 128], BF16, tag="onesrow")
        qscale = cp.tile([1, 128], BF16, tag="qscale")
        onesf32 = cp.tile([1, 128], F32, tag="onesf32")
        ones128 = cp.tile([128, 128], F32, tag="ones128")
        epscol = cp.tile([128, 1], F32, tag="epscol")
        for nm, t in [("wcat", wcat), ("wab", wab), ("wout", wout),
                      ("convw", convw), ("dtbnea", dtbnea), ("idf32", idf32),
                      ("idbf", idbf), ("penUi", penUi), ("penUs", penUs),
                      ("penLs", penLs), ("uincl", uincl), ("lstrict", lstrict),
                      ("onesbf", onesbf), ("onesrow", onesrow), ("qscale", qscale), ("onesf32", onesf32), ("ones128", ones128), ("epscol", epscol)]:
            nc.sync.dma_start(t[:], d[nm][:])

        # persistent state / raw-tail
        sp = ctx.enter_context(tc.tile_pool(name="state", bufs=1))
        S = sp.tile([128, 4, 128], BF16, tag="S")         # per-head states
        qkvraw = sp.tile([128, 8, BT + 3], BF16, tag="qkvraw")

        # per-block buffers
        bp = ctx.enter_context(tc.tile_pool(name="blk", bufs=1))
        xt = bp.tile([128, KT, BT], BF16, tag="xt")
        csil = bp.tile([128, 8, BT], BF16, tag="csil")    # conv+silu qkv
        qk = bp.tile([128, 4, BT], BF16, tag="qk")        # normed q(0,1) k(2,3)
        zs = bp.tile([128, 4, BT], BF16, tag="zs")        # silu(z)
        gT = bp.tile([128, 4, BT], BF16, tag="gT")        # gated transposed
        abf = bp.tile([8, BT], F32, tag="abf")
        abT = bp.tile([128, NCH, 8], F32, tag="abT")
        # column tensors [128, 16] (= NCH*4 heads)
        gcum = bp.tile([128, 16], F32, tag="gcum")
        ngcum = bp.tile([128, 16], F32, tag="ngcum")
        eg = bp.tile([128, 16], F32, tag="eg")
        etail = bp.tile([128, 16], F32, tag="etail")
        beta = bp.tile([128, 16], F32, tag="beta")
        nbeta = bp.tile([128, 16], F32, tag="nbeta")
        bg = bp.tile([128, 16], F32, tag="bg")
        ucol = bp.tile([128, 16], F32, tag="ucol")
        spb = bp.tile([128, 16], F32, tag="spb")
        spa = bp.tile([128, 16], F32, tag="spa")
        a2t = bp.tile([128, 16], F32, tag="a2t")
        elam = bp.tile([128, 16], F32, tag="elam")
        el0 = bp.tile([1, 16], F32, tag="el0")
        el127 = bp.tile([1, 16], F32, tag="el127")

        # rotating scratch
        wp = ctx.enter_context(tc.tile_pool(name="scr", bufs=2))
        w1 = ctx.enter_context(tc.tile_pool(name="scr1", bufs=1))
        w3 = ctx.enter_context(tc.tile_pool(name="scr3", bufs=3))
        wcv = ctx.enter_context(tc.tile_pool(name="scrcv", bufs=5))
        # psum pools
        pp = ctx.enter_context(tc.tile_pool(name="pp", bufs=2, space="PSUM"))
        pt1 = ctx.enter_context(tc.tile_pool(name="pt1", bufs=1, space="PSUM"))
        pcc = ctx.enter_context(tc.tile_pool(name="pcc", bufs=3, space="PSUM"))
        ptr = ctx.enter_context(tc.tile_pool(name="ptr", bufs=1, space="PSUM"))

        def mm(ps, lhsT, rhs, start, stop):
            nc.tensor.matmul(ps, lhsT, rhs, start=start, stop=stop)

        for blk in range(NBLK):
            t0 = blk * BT
            if blk % 8 == 0:
                nc.vector.memset(S[:], 0.0)
                nc.vector.memset(qkvraw[:, :, 0:3], 0.0)

            # ---- x load, cast, transpose -> xt[hid, tok]
            for tt in range(4):
                xin = w1.tile([128, HID], F32, tag="xin")
                nc.sync.dma_start(xin[:], d["x"][t0 + tt * 128: t0 + (tt + 1) * 128, :])
                xbf = wp.tile([128, HID], BF16, tag="xbf")
                nc.scalar.copy(xbf[:], xin[:])
                for kt in range(KT):
                    ps = ptr.tile([128, 128], BF16, tag="tr")
                    nc.tensor.transpose(ps[:], xbf[:, kt * 128:(kt + 1) * 128], idbf[:])
                    nc.vector.tensor_copy(xt[:, kt, tt * 128:(tt + 1) * 128], ps[:])

            if STAGE < 2:
                ysb0 = wp.tile([128, 512], F32, tag="ysb")
                nc.vector.tensor_copy(ysb0[:], xt[:, 0, :])
                nc.sync.dma_start(d["y"][t0: t0 + 128, 0:512], ysb0[:])
                continue
            # ---- projections: 8 qkv tiles -> raw, 4 z tiles -> silu
            for ft in range(12):
                ps = pp.tile([128, BT], F32, tag="pp")
                for kt in range(KT):
                    mm(ps[:], wcat[:, kt * 1536 + ft * 128: kt * 1536 + (ft + 1) * 128],
                       xt[:, kt, :], kt == 0, kt == KT - 1)
                if ft < 8:
                    nc.scalar.copy(qkvraw[:, ft, 3:3 + BT], ps[:])
                else:
                    zf = wp.tile([128, BT], F32, tag="zf")
                    nc.scalar.copy(zf[:], ps[:])
                    ze = wp.tile([128, BT], F32, tag="ze")
                    nc.scalar.activation(ze[:], zf[:], AF.Exp, scale=-1.0)
                    nc.scalar.activation(ze[:], ze[:], AF.Identity, bias=1.0)
                    nc.vector.reciprocal(ze[:], ze[:])
                    nc.vector.tensor_tensor(zs[:, ft - 8, :], zf[:], ze[:], AL.mult)

            # ---- ab projection [8, BT] + per-chunk transpose
            psab = pp.tile([8, BT], F32, tag="pp")
            for kt in range(KT):
                mm(psab[:], wab[:, kt * 8:(kt + 1) * 8], xt[:, kt, :], kt == 0, kt == KT - 1)
            nc.scalar.copy(abf[:], psab[:])
            for ch in range(NCH):
                pst = pcc.tile([128, 8], F32, tag="cc")
                nc.tensor.transpose(pst[:], abf[:, ch * 128:(ch + 1) * 128], idf32[0:8, 0:8])
                nc.scalar.copy(abT[:, ch, :], pst[:])

            if STAGE < 3:
                ysb0 = wp.tile([128, 515], F32, tag="ysbr")
                nc.vector.tensor_copy(ysb0[:], qkvraw[:, 0, :])
                nc.sync.dma_start(d["y"][t0: t0 + 128, 0:515], ysb0[:])
                continue
            # ---- conv (4 taps) + silu, per channel tile
            for ct in range(8):
                c1 = wcv.tile([128, BT], F32, tag="cv")
                nc.vector.tensor_scalar(c1[:], qkvraw[:, ct, 0:BT], convw[:, ct * 4 + 0: ct * 4 + 1], None, AL.mult)
                c2 = wcv.tile([128, BT], F32, tag="cv")
                nc.vector.scalar_tensor_tensor(c2[:], qkvraw[:, ct, 1:1 + BT], convw[:, ct * 4 + 1: ct * 4 + 2], c1[:], AL.mult, AL.add)
                c3 = wcv.tile([128, BT], F32, tag="cv")
                nc.vector.scalar_tensor_tensor(c3[:], qkvraw[:, ct, 2:2 + BT], convw[:, ct * 4 + 2: ct * 4 + 3], c2[:], AL.mult, AL.add)
                c4 = wcv.tile([128, BT], F32, tag="cv")
                nc.vector.scalar_tensor_tensor(c4[:], qkvraw[:, ct, 3:3 + BT], convw[:, ct * 4 + 3: ct * 4 + 4], c3[:], AL.mult, AL.add)
                ce = wcv.tile([128, BT], F32, tag="cv")
                nc.scalar.activation(ce[:], c4[:], AF.Exp, scale=-1.0)
                nc.scalar.activation(ce[:], ce[:], AF.Identity, bias=1.0)
                nc.vector.reciprocal(ce[:], ce[:])
                nc.vector.tensor_tensor(csil[:, ct, :], c4[:], ce[:], AL.mult)
            # carry raw tail to next block
            nc.vector.tensor_copy(qkvraw[:, :, 0:3], qkvraw[:, :, BT:BT + 3])
            if STAGE < 2.5:
                ysb0 = wp.tile([128, 512], F32, tag="ysb")
                nc.vector.tensor_copy(ysb0[:], csil[:, 0, :])
                nc.sync.dma_start(d["y"][t0: t0 + 128, 0:512], ysb0[:])
                continue

            # ---- l2norm on q,k tiles (ct 0,1=q; 2,3=k)
            for ct in range(4):
                sq = wp.tile([128, BT], BF16, tag="sq")
                nc.scalar.activation(sq[:], csil[:, ct, :], AF.Square)
                pssum = pcc.tile([1, BT], F32, tag="cc")
                mm(pssum[:], onesbf[:], sq[:], True, True)
                sr = wp.tile([1, BT], F32, tag="sr")
                nc.scalar.activation(sr[:], pssum[:], AF.Ln, bias=epscol[0:1, :])
                rib = wp.tile([1, BT], BF16, tag="rib")
                nc.scalar.activation(rib[:], sr[:], AF.Exp, scale=-0.5)
                psb = pcc.tile([128, BT], F32, tag="cc")
                mm(psb[:], qscale[:] if ct < 2 else onesrow[:], rib[:], True, True)
                nc.vector.tensor_tensor(qk[:, ct, :], csil[:, ct, :], psb[:], AL.mult)

            if STAGE < 2.8:
                ysb0 = wp.tile([128, 512], F32, tag="ysb")
                nc.vector.tensor_copy(ysb0[:], qk[:, 0, :])
                nc.sync.dma_start(d["y"][t0: t0 + 128, 0:512], ysb0[:])
                continue
            # ---- column pipeline
            braw = abT[:, :, 0:4]
            araw = abT[:, :, 4:8]
            ebt = wp.tile([128, 16], F32, tag="ebt")
            ebv = ebt[:].rearrange("p (c h) -> p c h", h=4)
            nc.scalar.activation(ebv, braw, AF.Exp, scale=-1.0)
            nc.scalar.activation(ebt[:], ebt[:], AF.Identity, bias=1.0)
            nc.vector.reciprocal(beta[:], ebt[:])
            nc.scalar.activation(spb[:], ebt[:], AF.Ln)
            a2v = a2t[:].rearrange("p (c h) -> p c h", h=4)
            nc.vector.tensor_tensor(a2v, araw, dtbnea[:, None, 0:4].to_broadcast([128, NCH, 4]), AL.add)
            eat = wp.tile([128, 16], F32, tag="eat")
            eav = eat[:].rearrange("p (c h) -> p c h", h=4)
            nc.scalar.activation(eav, a2v, AF.Exp)
            nc.scalar.activation(eat[:], eat[:], AF.Identity, bias=1.0)
            nc.scalar.activation(spa[:], eat[:], AF.Ln)
            graw = wp.tile([128, 16], F32, tag="graw")
            gv = graw[:].rearrange("p (c h) -> p c h", h=4)
            nc.vector.tensor_tensor(gv, spa[:].rearrange("p (c h) -> p c h", h=4), dtbnea[:, None, 4:8].to_broadcast([128, NCH, 4]), AL.mult)
            psg = pcc.tile([128, 16], F32, tag="cc")
            mm(psg[:], uincl[:], graw[:], True, True)
            nc.scalar.copy(gcum[:], psg[:])
            nc.scalar.activation(eg[:], psg[:], AF.Exp)
            pst_ = pcc.tile([128, 16], F32, tag="cc")
            mm(pst_[:], lstrict[:], graw[:], True, True)
            nc.scalar.activation(etail[:], pst_[:], AF.Exp)
            nc.vector.tensor_scalar(ngcum[:], gcum[:], -1.0, None, AL.mult)
            nc.vector.scalar_tensor_tensor(ucol[:], spb[:], -1.0, gcum[:], AL.mult, AL.add)
            nc.vector.tensor_tensor(bg[:], beta[:], eg[:], AL.mult)
            nc.vector.tensor_scalar(nbeta[:], beta[:], -1.0, None, AL.mult)
            # elast broadcast
            nc.sync.dma_start(el127[:], gcum[127:128, :])
            nc.scalar.activation(el0[:], el127[:], AF.Exp)
            psel = pcc.tile([128, 16], F32, tag="cc")
            mm(psel[:], onesf32[:], el0[:], True, True)
            nc.scalar.copy(elam[:], psel[:])

            if STAGE < 4:
                ysb0 = wp.tile([128, 512], F32, tag="ysb")
                nc.vector.tensor_copy(ysb0[:], qk[:, 0, :])
                nc.sync.dma_start(d["y"][t0: t0 + 128, 0:512], ysb0[:])
                continue
            # ---- chunks
            for ch in range(NCH):
                # batched T1 matrices for 4 heads
                dgG = w1.tile([128, 4, 128], F32, tag="dgG")
                nc.vector.tensor_tensor(dgG[:], gcum[:, ch * 4:(ch + 1) * 4, None].to_broadcast([128, 4, 128]),
                                        idf32[:, None, :].to_broadcast([128, 4, 128]), AL.mult)
                dgU = w1.tile([128, 4, 128], F32, tag="dgU")
                nc.vector.tensor_tensor(dgU[:], ucol[:, ch * 4:(ch + 1) * 4, None].to_broadcast([128, 4, 128]),
                                        idf32[:, None, :].to_broadcast([128, 4, 128]), AL.mult)
                t1g = pt1.tile([128, 512], F32, tag="t1g")
                mm(t1g[:], ones128[:], dgG[:].rearrange("p r q -> p (r q)"), True, True)
                t1u = pt1.tile([128, 512], F32, tag="t1u")
                mm(t1u[:], ones128[:], dgU[:].rearrange("p r q -> p (r q)"), True, True)
                t1gp = w1.tile([128, 4, 128], F32, tag="t1gp")
                nc.vector.tensor_tensor(t1gp[:].rearrange("p r q -> p (r q)"), t1g[:], penUi[:], AL.add)
                t1up = w1.tile([128, 4, 128], F32, tag="t1up")
                nc.vector.tensor_tensor(t1up[:].rearrange("p r q -> p (r q)"), t1u[:], penUs[:], AL.add)
                t1gl = w1.tile([128, 4, 128], F32, tag="t1gl")
                nc.vector.scalar_tensor_tensor(t1gl[:].rearrange("p r q -> p (r q)"), t1g[:], -1.0, penLs[:], AL.mult, AL.add)

                if STAGE < 3.4:
                    continue
                for hi in range(4):
                    col = ch * 4 + hi
                    kh = hi // 2
                    cs = slice(ch * 128, (ch + 1) * 128)
                    qT = qk[:, kh, cs]
                    kTt = qk[:, 2 + kh, cs]
                    vT = csil[:, 4 + hi, cs]
                    gc_c = gcum[:, col:col + 1]
                    ngc_c = ngcum[:, col:col + 1]

                    DTi = w3.tile([128, 128], BF16, tag="DTi")
                    nc.scalar.activation(DTi[:], t1gp[:, hi, :], AF.Exp, bias=ngc_c)
                    DTBs = w3.tile([128, 128], BF16, tag="DTBs")
                    nc.scalar.activation(DTBs[:], t1up[:, hi, :], AF.Exp, bias=ngc_c)
                    DBLs = w3.tile([128, 128], BF16, tag="DBLs")
                    nc.scalar.activation(DBLs[:], t1gl[:, hi, :], AF.Exp, bias=gc_c)

                    kps = ptr.tile([128, 128], BF16, tag="tr")
                    nc.tensor.transpose(kps[:], kTt, idbf[:])
                    ksb = w3.tile([128, 128], BF16, tag="ksb")
                    nc.vector.tensor_copy(ksb[:], kps[:])
                    vps = ptr.tile([128, 128], BF16, tag="tr")
                    nc.tensor.transpose(vps[:], vT, idbf[:])
                    vsb = w3.tile([128, 128], BF16, tag="vsb")
                    nc.vector.tensor_copy(vsb[:], vps[:])

                    m2 = pcc.tile([128, 128], F32, tag="cc")
                    mm(m2[:], kTt, kTt, True, True)
                    AT = w3.tile([128, 128], BF16, tag="AT")
                    nc.vector.scalar_tensor_tensor(AT[:], m2[:], -1.0, DTBs[:], AL.mult, AL.mult)
                    Am = w3.tile([128, 128], BF16, tag="Am")
                    nc.vector.scalar_tensor_tensor(Am[:], m2[:], nbeta[:, col:col + 1], DBLs[:], AL.mult, AL.mult)

                    if STAGE < 3.7:
                        continue
                    # solve: attnT = (I+N)(I+N^2)(I+N^4), N=AT
                    X1 = w3.tile([128, 128], BF16, tag="X1")
                    nc.vector.tensor_tensor(X1[:], AT[:], idbf[:], AL.add)
                    a2p = pcc.tile([128, 128], F32, tag="cc")
                    mm(a2p[:], AT[:], Am[:], True, True)
                    A2 = w3.tile([128, 128], BF16, tag="A2")
                    nc.scalar.copy(A2[:], a2p[:])
                    n2p = pcc.tile([128, 128], F32, tag="cc")
                    mm(n2p[:], Am[:], AT[:], True, True)
                    N2 = w3.tile([128, 128], BF16, tag="N2")
                    nc.scalar.copy(N2[:], n2p[:])
                    A2i = w3.tile([128, 128], BF16, tag="A2i")
                    nc.vector.tensor_tensor(A2i[:], A2[:], idbf[:], AL.add)
                    x2p = pcc.tile([128, 128], F32, tag="cc")
                    mm(x2p[:], A2i[:], X1[:], True, True)
                    X2 = w3.tile([128, 128], BF16, tag="X2")
                    nc.scalar.copy(X2[:], x2p[:])
                    a4p = pcc.tile([128, 128], F32, tag="cc")
                    mm(a4p[:], N2[:], A2[:], True, True)
                    A4i = w3.tile([128, 128], BF16, tag="A4i")
                    nc.scalar.copy(A4i[:], a4p[:])
                    nc.vector.tensor_tensor(A4i[:], A4i[:], idbf[:], AL.add)
                    x3p = pcc.tile([128, 128], F32, tag="cc")
                    mm(x3p[:], A4i[:], X2[:], True, True)
                    attnT = w3.tile([128, 128], BF16, tag="attnT")
                    nc.scalar.copy(attnT[:], x3p[:])

                    vb = w3.tile([128, 128], BF16, tag="vb")
                    nc.vector.tensor_scalar(vb[:], vsb[:], beta[:, col:col + 1], None, AL.mult)
                    kbg = w3.tile([128, 128], BF16, tag="kbg")
                    nc.vector.tensor_scalar(kbg[:], ksb[:], bg[:, col:col + 1], None, AL.mult)
                    ket = w3.tile([128, 128], BF16, tag="ket")
                    nc.vector.tensor_scalar(ket[:], ksb[:], etail[:, col:col + 1], None, AL.mult)

                    vcp = pcc.tile([128, 128], F32, tag="cc")
                    mm(vcp[:], attnT[:], vb[:], True, True)
                    vc = w3.tile([128, 128], BF16, tag="vc")
                    nc.scalar.copy(vc[:], vcp[:])
                    kcdp = pcc.tile([128, 128], F32, tag="cc")
                    mm(kcdp[:], kbg[:], attnT[:], True, True)
                    kcdT = w3.tile([128, 128], BF16, tag="kcdT")
                    nc.scalar.copy(kcdT[:], kcdp[:])
                    kqp = pcc.tile([128, 128], F32, tag="cc")
                    mm(kqp[:], kTt, qT, True, True)
                    atiT = w3.tile([128, 128], BF16, tag="atiT")
                    nc.vector.tensor_tensor(atiT[:], kqp[:], DTi[:], AL.mult)

                    o1p = pcc.tile([128, 128], F32, tag="cc")
                    mm(o1p[:], qT, S[:, hi, :], True, True)
                    wqp = pcc.tile([128, 128], F32, tag="cc")
                    mm(wqp[:], kcdT[:], S[:, hi, :], True, True)
                    vnew = w3.tile([128, 128], BF16, tag="vnew")
                    nc.vector.scalar_tensor_tensor(vnew[:], wqp[:], -1.0, vc[:], AL.mult, AL.add)
                    o2p = pcc.tile([128, 128], F32, tag="cc")
                    mm(o2p[:], atiT[:], vnew[:], True, True)
                    sup = pcc.tile([128, 128], F32, tag="cc")
                    mm(sup[:], ket[:], vnew[:], True, True)
                    og = w3.tile([128, 128], F32, tag="og")
                    nc.vector.tensor_scalar(og[:], o1p[:], eg[:, col:col + 1], None, AL.mult)
                    oc = w3.tile([128, 128], F32, tag="oc")
                    nc.vector.tensor_tensor(oc[:], og[:], o2p[:], AL.add)
                    nc.vector.scalar_tensor_tensor(S[:, hi, :], S[:, hi, :], elam[:, col:col + 1], sup[:], AL.mult, AL.add)

                    # gating: rms over free dim, transpose, * silu(z)
                    osq = w3.tile([128, 128], F32, tag="osq")
                    ss = w3.tile([128, 1], F32, tag="ss")
                    nc.scalar.activation(osq[:], oc[:], AF.Square, accum_out=ss[:])
                    sr2 = w3.tile([128, 1], F32, tag="sr2")
                    nc.scalar.activation(sr2[:], ss[:], AF.Ln, bias=epscol[:], scale=1.0 / 128.0)
                    ri2 = w3.tile([128, 1], F32, tag="ri2")
                    nc.scalar.activation(ri2[:], sr2[:], AF.Exp, scale=-0.5)
                    orm = w3.tile([128, 128], BF16, tag="orm")
                    nc.vector.tensor_scalar(orm[:], oc[:], ri2[:], None, AL.mult)
                    otp = ptr.tile([128, 128], BF16, tag="tr")
                    nc.tensor.transpose(otp[:], orm[:], idbf[:])
                    nc.vector.tensor_tensor(gT[:, hi, cs], otp[:], zs[:, hi, cs], AL.mult)

            if STAGE < 4:
                ysb0 = wp.tile([128, 512], F32, tag="ysb")
                nc.vector.tensor_copy(ysb0[:], qk[:, 0, :])
                nc.sync.dma_start(d["y"][t0: t0 + 128, 0:512], ysb0[:])
                continue
            # ---- out projection
            for tt in range(4):
                for dt_ in range(4):
                    yps = pp.tile([128, 512], F32, tag="pp")
                    for vt in range(4):
                        mm(yps[:], gT[:, vt, tt * 128:(tt + 1) * 128],
                           wout[:, vt * 2048 + dt_ * 512: vt * 2048 + (dt_ + 1) * 512], vt == 0, vt == 3)
                    ysb = wp.tile([128, 512], F32, tag="ysb")
                    nc.scalar.copy(ysb[:], yps[:])
                    nc.sync.dma_start(d["y"][t0 + tt * 128: t0 + (tt + 1) * 128, dt_ * 512:(dt_ + 1) * 512], ysb[:])

    nc.compile()
    return nc


def make_inmaps(inp):
    import ml_dtypes
    bf = ml_dtypes.bfloat16
    x = np.ascontiguousarray(inp["x"].reshape(8192, HID).astype(np.float32))
    ii = np.arange(128)
    ident = np.eye(128, dtype=np.float32)
    idf32 = ident
    penUi = np.tile(np.where(ii[None, :] >= ii[:, None], 0, -1e9).astype(np.float32), (1, 4))
    penUs = np.tile(np.where(ii[None, :] > ii[:, None], 0, -1e9).astype(np.float32), (1, 4))
    penLs = np.tile(np.where(ii[:, None] > ii[None, :], 0, -1e9).astype(np.float32), (1, 4))
    uinclm = (ii[:, None] <= ii[None, :]).astype(np.float32)
    lstrictm = (ii[:, None] > ii[None, :]).astype(np.float32)
    maps = []
    for c in range(8):
        W = np.concatenate([inp["w_qkv"][:, 256 * c:256 * c + 256],
                            inp["w_qkv"][:, 2048 + 256 * c:2048 + 256 * c + 256],
                            inp["w_qkv"][:, 4096 + 512 * c:4096 + 512 * c + 512],
                            inp["w_z"][:, 512 * c:512 * c + 512]], 1)
        wcat = W.reshape(16, 128, 1536).transpose(1, 0, 2).reshape(128, -1).astype(bf)
        wab_ = np.concatenate([inp["w_b"][:, 4 * c:4 * c + 4], inp["w_a"][:, 4 * c:4 * c + 4]], 1)
        wab = wab_.reshape(16, 128, 8).transpose(1, 0, 2).reshape(128, -1).astype(bf)
        nwr = np.tile(inp["norm_w"], 4)
        wo = (inp["w_out"][512 * c:512 * c + 512] * nwr[:, None]).astype(np.float32)
        wout = wo.reshape(4, 128, 2048).transpose(1, 0, 2).reshape(128, -1).astype(bf)
        cw = np.concatenate([inp["conv_w"][256 * c:256 * c + 256],
                             inp["conv_w"][2048 + 256 * c:2048 + 256 * c + 256],
                             inp["conv_w"][4096 + 512 * c:4096 + 512 * c + 512]], 0)
        convw = cw.reshape(8, 128, 4).transpose(1, 0, 2).reshape(128, 32).astype(np.float32)
        dtbnea = np.zeros((128, 8), np.float32)
        dtbnea[:, 0:4] = inp["dt_bias"][4 * c:4 * c + 4][None, :]
        dtbnea[:, 4:8] = -np.exp(inp["a_log"][4 * c:4 * c + 4])[None, :]
        maps.append({
            "x": x, "wcat": wcat, "wab": wab, "wout": wout, "convw": convw,
            "dtbnea": dtbnea, "idf32": idf32,
            "idbf": ident.astype(bf), "penUi": penUi, "penUs": penUs,
            "penLs": penLs, "uincl": uinclm, "lstrict": lstrictm,
            "onesbf": np.ones((128, 1), bf), "onesrow": np.ones((1, 128), bf), "qscale": np.full((1, 128), 128 ** -0.5, bf),
            "onesf32": np.ones((1, 128), np.float32),
            "ones128": np.ones((128, 128), np.float32), "epscol": np.full((128, 1), 1e-6, np.float32),
        })
    return maps


_NC = None


def kernel(**inputs):
    """Full-input GatedDeltaNet forward on 8 NeuronCores (head-parallel)."""
    global _NC
    from concourse.bass_utils import run_bass_kernel_spmd
    inp = {k: np.asarray(v) for k, v in inputs.items()}
    maps = make_inmaps(inp)
    if _NC is None:
        _NC = build_nc()
    res = run_bass_kernel_spmd(_NC, maps, core_ids=list(range(8)))
    y = np.zeros((8192, HID), np.float64)
    for c in range(8):
        y += res.results[c]["y"].astype(np.float64)
    return y.astype(np.float32).reshape(2, 4096, HID)



# revision 18
# speedup vs baseline: 1.0123x; 1.0123x over previous
"""Bass kernel builder for GatedDeltaNet — 8-core SPMD, head-parallel.

Per core: 4 v-heads, 2 k-heads. Projections produce [feature, token]
(feature on partitions); delta-rule chunk math at C=128 with token-on-partition
tiles produced by DMA XBAR transposes; truncated Neumann-product triangular
solve (3 factors). Scalar-engine activations stay within one table
(exp/tanh/square/identity/copy) except two compact Ln clusters per block.
"""
import numpy as np
import concourse.bass as bass
import concourse.mybir as mybir
import concourse.tile as tile
from concourse import bacc
from contextlib import ExitStack

F32 = mybir.dt.float32
BF16 = mybir.dt.bfloat16
AL = mybir.AluOpType
AF = mybir.ActivationFunctionType

HID = 2048
C = 128          # chunk length
BT = 512         # token block
NBLK = 16        # 2 batches x 8 blocks
NCH = 4          # chunks per block
KT = 16          # hidden k-tiles
EPS = 1e-6


def build_nc():
    nc = bacc.Bacc("TRN2", target_bir_lowering=False, debug=False, num_devices=8)
    d = {}
    d["x"] = nc.dram_tensor("x", (8192, HID), F32, kind="ExternalInput")
    d["wcat"] = nc.dram_tensor("wcat", (128, KT * 1536), BF16, kind="ExternalInput")
    d["wab"] = nc.dram_tensor("wab", (128, KT * 8), BF16, kind="ExternalInput")
    d["wout"] = nc.dram_tensor("wout", (128, 4 * 2048), BF16, kind="ExternalInput")
    d["convd"] = nc.dram_tensor("convd", (128, 32 * 128), BF16, kind="ExternalInput")
    d["dtbnea"] = nc.dram_tensor("dtbnea", (128, 8), F32, kind="ExternalInput")
    d["idf32"] = nc.dram_tensor("idf32", (128, 128), F32, kind="ExternalInput")
    d["idbf"] = nc.dram_tensor("idbf", (128, 128), BF16, kind="ExternalInput")
    d["penUi"] = nc.dram_tensor("penUi", (128, 128), F32, kind="ExternalInput")
    d["penUs"] = nc.dram_tensor("penUs", (128, 128), F32, kind="ExternalInput")
    d["penLs"] = nc.dram_tensor("penLs", (128, 128), F32, kind="ExternalInput")
    d["uincl"] = nc.dram_tensor("uincl", (128, 128), F32, kind="ExternalInput")
    d["lstrict"] = nc.dram_tensor("lstrict", (128, 128), F32, kind="ExternalInput")
    d["onesbf"] = nc.dram_tensor("onesbf", (128, 1), BF16, kind="ExternalInput")
    d["onesrow"] = nc.dram_tensor("onesrow", (1, 128), BF16, kind="ExternalInput")
    d["qscale"] = nc.dram_tensor("qscale", (1, 128), BF16, kind="ExternalInput")
    d["onesf32"] = nc.dram_tensor("onesf32", (1, 128), F32, kind="ExternalInput")
    d["ones128"] = nc.dram_tensor("ones128", (128, 128), F32, kind="ExternalInput")
    d["epscol"] = nc.dram_tensor("epscol", (128, 1), F32, kind="ExternalInput")
    d["y"] = nc.dram_tensor("y", (8192, HID), F32, kind="ExternalOutput")

    with tile.TileContext(nc) as tc, ExitStack() as ctx:
        cp = ctx.enter_context(tc.tile_pool(name="consts", bufs=1))
        wcat = cp.tile([128, KT * 1536], BF16, tag="wcat")
        wab = cp.tile([128, KT * 8], BF16, tag="wab")
        wout = cp.tile([128, 4 * 2048], BF16, tag="wout")
        convd = cp.tile([128, 32 * 128], BF16, tag="convd")
        dtbnea = cp.tile([128, 8], F32, tag="dtbnea")
        idf32 = cp.tile([128, 128], F32, tag="idf32")
        idbf = cp.tile([128, 128], BF16, tag="idbf")
        penUi = cp.tile([128, 128], F32, tag="penUi")
        penUs = cp.tile([128, 128], F32, tag="penUs")
        penLs = cp.tile([128, 128], F32, tag="penLs")
        uincl = cp.tile([128, 128], F32, tag="uincl")
        lstrict = cp.tile([128, 128], F32, tag="lstrict")
        onesbf = cp.tile([128, 1], BF16, tag="onesbf")
        onesrow = cp.tile([1, 128], BF16, tag="onesrow")
        qscale = cp.tile([1, 128], BF16, tag="qscale")
        onesf32 = cp.tile([1, 128], F32, tag="onesf32")
        ones128 = cp.tile([128, 128], F32, tag="ones128")
        epscol = cp.tile([128, 1], F32, tag="epscol")
        for nm, t in [("wcat", wcat), ("wab", wab), ("wout", wout),
                      ("convd", convd), ("dtbnea", dtbnea), ("idf32", idf32),
                      ("idbf", idbf),
                      ("penUi", penUi), ("penUs", penUs), ("penLs", penLs),
                      ("uincl", uincl), ("lstrict", lstrict),
                      ("onesbf", onesbf), ("onesrow", onesrow),
                      ("qscale", qscale), ("onesf32", onesf32),
                      ("ones128", ones128), ("epscol", epscol)]:
            nc.sync.dma_start(t[:], d[nm][:])

        # persistent state
        sp = ctx.enter_context(tc.tile_pool(name="state", bufs=1))
        S = sp.tile([128, 4, 128], BF16, tag="S")
        tail = sp.tile([128, 8, 3], BF16, tag="tail")

        # per-block rotating pools
        fe = ctx.enter_context(tc.tile_pool(name="fe", bufs=2))
        w3 = ctx.enter_context(tc.tile_pool(name="scr3", bufs=2))
        # psum pools
        pp = ctx.enter_context(tc.tile_pool(name="pp", bufs=2, space="PSUM"))
        pt1 = ctx.enter_context(tc.tile_pool(name="pt1", bufs=1, space="PSUM"))
        pcc = ctx.enter_context(tc.tile_pool(name="pcc", bufs=3, space="PSUM"))

        def mm(ps, lhsT, rhs, start, stop):
            nc.tensor.matmul(ps, lhsT, rhs, start=start, stop=stop)

        for blk in range(NBLK):
            t0 = blk * BT
            if blk % 8 == 0:
                nc.vector.memset(S[:], 0.0)
                nc.gpsimd.memset(tail[:], 0.0)

            # ---- x load, gpsimd cast, DMA-XBAR transpose -> xt[hid128, kt, tok]
            xt = fe.tile([128, KT, BT], BF16, tag="xt")
            for tt in range(4):
                xbf = fe.tile([128, HID], BF16, tag="xbf")
                nc.gpsimd.dma_start(xbf[:], d["x"][t0 + tt * 128: t0 + (tt + 1) * 128, :])
                nc.sync.dma_start_transpose(xt[:, :, tt * 128:(tt + 1) * 128], xbf[:])

            # ---- projections: 8 qkv tiles -> raw, 4 z tiles -> tanh-silu
            qkvb = fe.tile([128, 8, BT + 3], BF16, tag="qkvb")
            nc.gpsimd.tensor_copy(qkvb[:, :, 0:3], tail[:])
            zs = fe.tile([128, 4, BT], BF16, tag="zs")
            for ft in range(12):
                ps = pp.tile([128, BT], F32, tag="pp")
                for kt in range(KT):
                    mm(ps[:], wcat[:, kt * 1536 + ft * 128: kt * 1536 + (ft + 1) * 128],
                       xt[:, kt, :], kt == 0, kt == KT - 1)
                if ft < 8:
                    if ft % 2 == 0:
                        nc.scalar.copy(qkvb[:, ft, 3:3 + BT], ps[:])
                    else:
                        nc.vector.tensor_copy(qkvb[:, ft, 3:3 + BT], ps[:])
                else:
                    th = w3.tile([128, BT], BF16, tag="th")
                    nc.scalar.activation(th[:], ps[:], AF.Tanh, scale=0.5)
                    # zs = 2*silu(z) = z*(1+tanh(z/2)); 0.5 folded into wout
                    nc.vector.scalar_tensor_tensor(zs[:, ft - 8, :], th[:], 1.0, ps[:], AL.add, AL.mult)

            # ---- ab projection [8, BT] + per-chunk transpose (f32, PE)
            psab = pp.tile([8, BT], F32, tag="pp")
            for kt in range(KT):
                mm(psab[:], wab[:, kt * 8:(kt + 1) * 8], xt[:, kt, :], kt == 0, kt == KT - 1)
            abf = fe.tile([8, BT], F32, tag="abf", bufs=1)
            nc.scalar.copy(abf[:], psab[:])
            abT = fe.tile([128, NCH, 8], F32, tag="abT")
            for ch in range(NCH):
                pst = pcc.tile([128, 8], F32, tag="cc")
                nc.tensor.transpose(pst[:], abf[:, ch * 128:(ch + 1) * 128], idf32[0:8, 0:8])
                nc.scalar.copy(abT[:, ch, :], pst[:])

            # ---- conv on PE (4 diag-stationary matmuls) + tanh-silu
            csil = fe.tile([128, 8, BT], BF16, tag="csil")
            for ct in range(8):
                cps = pp.tile([128, BT], F32, tag="pp")
                for tap in range(4):
                    mm(cps[:], convd[:, (ct * 4 + tap) * 128:(ct * 4 + tap + 1) * 128],
                       qkvb[:, ct, tap:tap + BT], tap == 0, tap == 3)
                cth = w3.tile([128, BT], BF16, tag="th")
                nc.scalar.activation(cth[:], cps[:], AF.Tanh, scale=0.5)
                # csil = 2*silu(conv); factor cancels in l2norm (q,k) / rmsnorm (v)
                nc.vector.scalar_tensor_tensor(csil[:, ct, :], cth[:], 1.0, cps[:], AL.add, AL.mult)
            # carry raw tail to next block
            nc.gpsimd.tensor_copy(tail[:], qkvb[:, :, BT:BT + 3])

            # ---- l2norm sums (q:0,1  k:2,3) -> ssq rows
            pssums = []
            for ct in range(4):
                sq = w3.tile([128, BT], BF16, tag="sq")
                nc.scalar.activation(sq[:], csil[:, ct, :], AF.Square)
                pssum = pcc.tile([1, BT], F32, tag="cc")
                mm(pssum[:], onesbf[:], sq[:], True, True)
                pssums.append(pssum)

            # ---- column pipeline (pre-cluster id0 ops)
            braw = abT[:, :, 0:4]
            araw = abT[:, :, 4:8]
            tb = fe.tile([128, 16], F32, tag="tb")
            nc.scalar.activation(tb[:].rearrange("p (c h) -> p c h", h=4), braw, AF.Tanh, scale=0.5)
            beta = fe.tile([128, 16], F32, tag="beta")
            nc.vector.tensor_scalar(beta[:], tb[:], 0.5, 0.5, AL.mult, AL.add)
            a2t = fe.tile([128, 16], F32, tag="a2t")
            nc.vector.tensor_tensor(a2t[:].rearrange("p (c h) -> p c h", h=4), araw,
                                    dtbnea[:, None, 0:4].to_broadcast([128, NCH, 4]), AL.add)
            ea = fe.tile([128, 16], F32, tag="ea")
            nc.scalar.activation(ea[:], a2t[:], AF.Exp)

            # ==== Ln cluster 1 (one table load) ====
            for ct in range(4):
                nc.scalar.activation(pssums[ct][:], pssums[ct][:], AF.Ln, bias=epscol[0:1, :])
            lnbeta = fe.tile([128, 16], F32, tag="lnbeta")
            nc.scalar.activation(lnbeta[:], beta[:], AF.Ln)
            spa = fe.tile([128, 16], F32, tag="spa")
            nc.scalar.activation(spa[:], ea[:], AF.Ln, bias=1.0)

            # ==== back to exp table ====
            ribs = []
            for ct in range(4):
                rib = fe.tile([1, BT], BF16, tag=f"rib{ct}", bufs=1)
                nc.scalar.activation(rib[:], pssums[ct][:], AF.Exp, scale=-0.5)
                ribs.append(rib)
            for ct in range(4):
                psb = pcc.tile([128, BT], F32, tag="cc")
                mm(psb[:], qscale[:] if ct < 2 else onesrow[:], ribs[ct][0:1, :], True, True)
                nc.vector.tensor_tensor(csil[:, ct, :], csil[:, ct, :], psb[:], AL.mult)

            graw = fe.tile([128, 16], F32, tag="graw")
            nc.vector.tensor_tensor(graw[:].rearrange("p (c h) -> p c h", h=4),
                                    spa[:].rearrange("p (c h) -> p c h", h=4),
                                    dtbnea[:, None, 4:8].to_broadcast([128, NCH, 4]), AL.mult)
            psg = pcc.tile([128, 16], F32, tag="cc")
            mm(psg[:], uincl[:], graw[:], True, True)
            gcum = fe.tile([128, 16], F32, tag="gcum")
            nc.scalar.copy(gcum[:], psg[:])
            eg = fe.tile([128, 16], F32, tag="eg")
            nc.scalar.activation(eg[:], psg[:], AF.Exp)
            pst_ = pcc.tile([128, 16], F32, tag="cc")
            mm(pst_[:], lstrict[:], graw[:], True, True)
            etail = fe.tile([128, 16], F32, tag="etail")
            nc.scalar.activation(etail[:], pst_[:], AF.Exp)
            ngcum = fe.tile([128, 16], F32, tag="ngcum")
            nc.vector.tensor_scalar(ngcum[:], gcum[:], -1.0, None, AL.mult)
            ucol = fe.tile([128, 16], F32, tag="ucol")
            nc.vector.tensor_tensor(ucol[:], gcum[:], lnbeta[:], AL.add)
            betah = fe.tile([128, 16], F32, tag="betah")
            nc.gpsimd.tensor_scalar(betah[:], beta[:], 0.5, None, AL.mult)
            bg = fe.tile([128, 16], F32, tag="bg")
            nc.gpsimd.tensor_tensor(bg[:], beta[:], eg[:], AL.mult)
            nbeta = fe.tile([128, 16], F32, tag="nbeta")
            nc.gpsimd.tensor_scalar(nbeta[:], beta[:], -1.0, None, AL.mult)
            el127 = fe.tile([1, 16], F32, tag="el127", bufs=1)
            nc.sync.dma_start(el127[:], gcum[127:128, :])
            el0 = fe.tile([1, 16], F32, tag="el0")
            nc.scalar.activation(el0[:], el127[:], AF.Exp)
            psel = pcc.tile([128, 16], F32, tag="cc")
            mm(psel[:], onesf32[:], el0[:], True, True)
            elam = fe.tile([128, 16], F32, tag="elam")
            nc.scalar.copy(elam[:], psel[:])

            # ---- chunks
            oc = fe.tile([128, 16, 128], BF16, tag="oc", bufs=1)
            sscol = fe.tile([128, 16], F32, tag="sscol")
            for ch in range(NCH):
                dgG = w3.tile([128, 4, 128], F32, tag="dgG", bufs=1)
                nc.gpsimd.tensor_tensor(dgG[:], gcum[:, ch * 4:(ch + 1) * 4, None].to_broadcast([128, 4, 128]),
                                        idf32[:, None, :].to_broadcast([128, 4, 128]), AL.mult)
                dgU = w3.tile([128, 4, 128], F32, tag="dgU", bufs=1)
                nc.gpsimd.tensor_tensor(dgU[:], ucol[:, ch * 4:(ch + 1) * 4, None].to_broadcast([128, 4, 128]),
                                        idf32[:, None, :].to_broadcast([128, 4, 128]), AL.mult)
                dgN = w3.tile([128, 4, 128], F32, tag="dgN", bufs=1)
                nc.gpsimd.tensor_tensor(dgN[:], ngcum[:, ch * 4:(ch + 1) * 4, None].to_broadcast([128, 4, 128]),
                                        idf32[:, None, :].to_broadcast([128, 4, 128]), AL.mult)
                # t1 matrices built fully in PSUM: broadcast matmul + accumulated pen mask
                t1gp = pt1.tile([128, 4, 128], F32, tag="t1g")
                mm(t1gp[:].rearrange("p r q -> p (r q)"), ones128[:], dgG[:].rearrange("p r q -> p (r q)"), True, False)
                for r in range(4):
                    mm(t1gp[:, r, :], idf32[:], penUi[:], False, r == 3)
                t1up = pt1.tile([128, 4, 128], F32, tag="t1u")
                mm(t1up[:].rearrange("p r q -> p (r q)"), ones128[:], dgU[:].rearrange("p r q -> p (r q)"), True, False)
                for r in range(4):
                    mm(t1up[:, r, :], idf32[:], penUs[:], False, r == 3)
                t1gl = pt1.tile([128, 4, 128], F32, tag="t1l")
                mm(t1gl[:].rearrange("p r q -> p (r q)"), ones128[:], dgN[:].rearrange("p r q -> p (r q)"), True, False)
                for r in range(4):
                    mm(t1gl[:, r, :], idf32[:], penLs[:], False, r == 3)

                for hi in range(4):
                    col = ch * 4 + hi
                    kh = hi // 2
                    cs = slice(ch * 128, (ch + 1) * 128)
                    qT = csil[:, kh, cs]
                    kTt = csil[:, 2 + kh, cs]
                    vT = csil[:, 4 + hi, cs]
                    gc_c = gcum[:, col:col + 1]
                    ngc_c = ngcum[:, col:col + 1]

                    # token-on-partition copies via DMA XBAR
                    ksb = w3.tile([128, 128], BF16, tag="ksb")
                    nc.sync.dma_start_transpose(ksb[:], kTt)
                    vsb = w3.tile([128, 128], BF16, tag="vsb")
                    nc.sync.dma_start_transpose(vsb[:], vT)

                    DTi = w3.tile([128, 128], BF16, tag="DTi")
                    nc.scalar.activation(DTi[:], t1gp[:, hi, :], AF.Exp, bias=ngc_c)
                    DTBs = w3.tile([128, 128], BF16, tag="DTBs")
                    nc.scalar.activation(DTBs[:], t1up[:, hi, :], AF.Exp, bias=ngc_c)
                    DBLs = w3.tile([128, 128], BF16, tag="DBLs")
                    nc.scalar.activation(DBLs[:], t1gl[:, hi, :], AF.Exp, bias=gc_c)

                    m2 = pcc.tile([128, 128], F32, tag="cc")
                    mm(m2[:], kTt, kTt, True, True)
                    AT = w3.tile([128, 128], BF16, tag="AT")
                    nc.vector.scalar_tensor_tensor(AT[:], m2[:], -1.0, DTBs[:], AL.mult, AL.mult)
                    Am = w3.tile([128, 128], BF16, tag="Am")
                    nc.vector.scalar_tensor_tensor(Am[:], m2[:], nbeta[:, col:col + 1], DBLs[:], AL.mult, AL.mult)

                    # solve: attnT = (I+N)(I+N^2)(I+N^4), N=AT
                    X1 = w3.tile([128, 128], BF16, tag="X1")
                    nc.gpsimd.tensor_tensor(X1[:], AT[:], idbf[:], AL.add)
                    a2p = pcc.tile([128, 128], F32, tag="cc")
                    mm(a2p[:], AT[:], Am[:], True, True)
                    A2 = w3.tile([128, 128], BF16, tag="A2")
                    nc.scalar.copy(A2[:], a2p[:])
                    A2i = w3.tile([128, 128], BF16, tag="A2i")
                    nc.vector.scalar_tensor_tensor(A2i[:], a2p[:], 1.0, idf32[:], AL.mult, AL.add)
                    n2p = pcc.tile([128, 128], F32, tag="cc")
                    mm(n2p[:], Am[:], AT[:], True, True)
                    N2 = w3.tile([128, 128], BF16, tag="N2")
                    nc.scalar.copy(N2[:], n2p[:])
                    x2p = pcc.tile([128, 128], F32, tag="cc")
                    mm(x2p[:], A2i[:], X1[:], True, True)
                    X2 = w3.tile([128, 128], BF16, tag="X2")
                    nc.scalar.copy(X2[:], x2p[:])
                    a4p = pcc.tile([128, 128], F32, tag="cc")
                    mm(a4p[:], N2[:], A2[:], True, True)
                    A4i = w3.tile([128, 128], BF16, tag="A4i")
                    nc.vector.scalar_tensor_tensor(A4i[:], a4p[:], 1.0, idf32[:], AL.mult, AL.add)
                    x3p = pcc.tile([128, 128], F32, tag="cc")
                    mm(x3p[:], A4i[:], X2[:], True, True)
                    attnT = w3.tile([128, 128], BF16, tag="attnT")
                    nc.scalar.copy(attnT[:], x3p[:])

                    vb = w3.tile([128, 128], BF16, tag="vb")
                    nc.gpsimd.tensor_scalar(vb[:], vsb[:], betah[:, col:col + 1], None, AL.mult)
                    kbg = w3.tile([128, 128], BF16, tag="kbg")
                    nc.gpsimd.tensor_scalar(kbg[:], ksb[:], bg[:, col:col + 1], None, AL.mult)
                    ket = w3.tile([128, 128], BF16, tag="ket")
                    nc.gpsimd.tensor_scalar(ket[:], ksb[:], etail[:, col:col + 1], None, AL.mult)

                    vcp = pcc.tile([128, 128], F32, tag="cc")
                    mm(vcp[:], attnT[:], vb[:], True, True)
                    vc = w3.tile([128, 128], BF16, tag="vc")
                    nc.vector.tensor_copy(vc[:], vcp[:])
                    kcdp = pcc.tile([128, 128], F32, tag="cc")
                    mm(kcdp[:], kbg[:], attnT[:], True, True)
                    kcdT = w3.tile([128, 128], BF16, tag="kcdT")
                    nc.scalar.copy(kcdT[:], kcdp[:])
                    kqp = pcc.tile([128, 128], F32, tag="cc")
                    mm(kqp[:], kTt, qT, True, True)
                    atiT = w3.tile([128, 128], BF16, tag="atiT")
                    nc.vector.tensor_tensor(atiT[:], kqp[:], DTi[:], AL.mult)

                    o1p = pcc.tile([128, 128], F32, tag="cc")
                    mm(o1p[:], qT, S[:, hi, :], True, True)
                    wqp = pcc.tile([128, 128], F32, tag="cc")
                    mm(wqp[:], kcdT[:], S[:, hi, :], True, True)
                    vnew = w3.tile([128, 128], BF16, tag="vnew")
                    nc.vector.scalar_tensor_tensor(vnew[:], wqp[:], -1.0, vc[:], AL.mult, AL.add)
                    o2p = pcc.tile([128, 128], F32, tag="cc")
                    mm(o2p[:], atiT[:], vnew[:], True, True)
                    sup = pcc.tile([128, 128], F32, tag="cc")
                    mm(sup[:], ket[:], vnew[:], True, True)
                    # oc = o1p*eg + o2p  (token-on-partition, [128,128])
                    og = w3.tile([128, 128], F32, tag="og")
                    nc.vector.tensor_scalar(og[:], o1p[:], eg[:, col:col + 1], None, AL.mult)
                    nc.vector.tensor_tensor(oc[:, col, :], og[:], o2p[:], AL.add)
                    nc.vector.scalar_tensor_tensor(S[:, hi, :], S[:, hi, :], elam[:, col:col + 1], sup[:], AL.mult, AL.add)
                    osq = w3.tile([128, 128], BF16, tag="osq")
                    nc.scalar.activation(osq[:], oc[:, col, :], AF.Square, accum_out=sscol[:, col:col + 1])

            # ==== Ln cluster 2: batched rms over the whole block ====
            lnr = fe.tile([128, 16], F32, tag="lnr")
            nc.scalar.activation(lnr[:], sscol[:], AF.Ln, scale=1.0 / 128.0, bias=epscol[:])
            rir = fe.tile([128, 16], F32, tag="rir")
            nc.scalar.activation(rir[:], lnr[:], AF.Exp, scale=-0.5)

            # ---- gating + out projection
            gT = fe.tile([128, 4, BT], BF16, tag="gT", bufs=1)
            for ch in range(NCH):
                for hi in range(4):
                    col = ch * 4 + hi
                    cs = slice(ch * 128, (ch + 1) * 128)
                    orm = w3.tile([128, 128], BF16, tag="orm")
                    if hi % 2 == 0:
                        nc.vector.tensor_scalar(orm[:], oc[:, col, :], rir[:, col:col + 1], None, AL.mult)
                    else:
                        nc.gpsimd.tensor_scalar(orm[:], oc[:, col, :], rir[:, col:col + 1], None, AL.mult)
                    otT = w3.tile([128, 128], BF16, tag="otT")
                    nc.sync.dma_start_transpose(otT[:], orm[:])
                    if hi % 2 == 0:
                        nc.gpsimd.tensor_tensor(gT[:, hi, cs], otT[:], zs[:, hi, cs], AL.mult)
                    else:
                        nc.vector.tensor_tensor(gT[:, hi, cs], otT[:], zs[:, hi, cs], AL.mult)

            for tt in range(4):
                for dt_ in range(4):
                    yps = pp.tile([128, 512], F32, tag="pp")
                    for vt in range(4):
                        mm(yps[:], gT[:, vt, tt * 128:(tt + 1) * 128],
                           wout[:, vt * 2048 + dt_ * 512: vt * 2048 + (dt_ + 1) * 512], vt == 0, vt == 3)
                    ysb = w3.tile([128, 512], F32, tag="ysb")
                    if dt_ % 2 == 0:
                        nc.scalar.copy(ysb[:], yps[:])
                    else:
                        nc.vector.tensor_copy(ysb[:], yps[:])
                    nc.sync.dma_start(d["y"][t0 + tt * 128: t0 + (tt + 1) * 128, dt_ * 512:(dt_ + 1) * 512], ysb[:])

    nc.compile()
    return nc


def make_inmaps(inp):
    import ml_dtypes
    bf = ml_dtypes.bfloat16
    x = np.ascontiguousarray(inp["x"].reshape(8192, HID).astype(np.float32))
    ii = np.arange(128)
    ident = np.eye(128, dtype=np.float32)
    penUi = np.where(ii[None, :] >= ii[:, None], 0, -1e9).astype(np.float32)
    penUs = np.where(ii[None, :] > ii[:, None], 0, -1e9).astype(np.float32)
    penLs = np.where(ii[:, None] > ii[None, :], 0, -1e9).astype(np.float32)
    uinclm = (ii[:, None] <= ii[None, :]).astype(np.float32)
    lstrictm = (ii[:, None] > ii[None, :]).astype(np.float32)
    maps = []
    for c in range(8):
        W = np.concatenate([inp["w_qkv"][:, 256 * c:256 * c + 256],
                            inp["w_qkv"][:, 2048 + 256 * c:2048 + 256 * c + 256],
                            inp["w_qkv"][:, 4096 + 512 * c:4096 + 512 * c + 512],
                            inp["w_z"][:, 512 * c:512 * c + 512]], 1)
        wcat = W.reshape(16, 128, 1536).transpose(1, 0, 2).reshape(128, -1).astype(bf)
        wab_ = np.concatenate([inp["w_b"][:, 4 * c:4 * c + 4], inp["w_a"][:, 4 * c:4 * c + 4]], 1)
        wab = wab_.reshape(16, 128, 8).transpose(1, 0, 2).reshape(128, -1).astype(bf)
        nwr = np.tile(inp["norm_w"], 4)
        # 0.5 fold: zs on-device is 2*silu(z)
        wo = (inp["w_out"][512 * c:512 * c + 512] * (0.5 * nwr)[:, None]).astype(np.float32)
        wout = wo.reshape(4, 128, 2048).transpose(1, 0, 2).reshape(128, -1).astype(bf)
        cw = np.concatenate([inp["conv_w"][256 * c:256 * c + 256],
                             inp["conv_w"][2048 + 256 * c:2048 + 256 * c + 256],
                             inp["conv_w"][4096 + 512 * c:4096 + 512 * c + 512]], 0)
        cwr = cw.reshape(8, 128, 4)  # [ct, feat, tap]
        convd = np.zeros((128, 32 * 128), np.float32)
        for ct in range(8):
            for tap in range(4):
                blk = (ct * 4 + tap) * 128
                convd[:, blk:blk + 128] = np.diag(cwr[ct, :, tap])
        dtbnea = np.zeros((128, 8), np.float32)
        dtbnea[:, 0:4] = inp["dt_bias"][4 * c:4 * c + 4][None, :]
        dtbnea[:, 4:8] = -np.exp(inp["a_log"][4 * c:4 * c + 4])[None, :]
        maps.append({
            "x": x, "wcat": wcat, "wab": wab, "wout": wout,
            "convd": convd.astype(bf), "dtbnea": dtbnea, "idf32": ident,
            "idbf": ident.astype(bf),
            "penUi": penUi, "penUs": penUs, "penLs": penLs,
            "uincl": uinclm, "lstrict": lstrictm,
            "onesbf": np.ones((128, 1), bf), "onesrow": np.ones((1, 128), bf),
            "qscale": np.full((1, 128), 128 ** -0.5, bf),
            "onesf32": np.ones((1, 128), np.float32),
            "ones128": np.ones((128, 128), np.float32),
            "epscol": np.full((128, 1), 1e-6, np.float32),
        })
    return maps


_NC = None


def kernel(**inputs):
    """Full-input GatedDeltaNet forward on 8 NeuronCores (head-parallel)."""
    global _NC
    from concourse.bass_utils import run_bass_kernel_spmd
    inp = {k: np.asarray(v) for k, v in inputs.items()}
    maps = make_inmaps(inp)
    if _NC is None:
        _NC = build_nc()
    res = run_bass_kernel_spmd(_NC, maps, core_ids=list(range(8)))
    y = np.zeros((8192, HID), np.float64)
    for c in range(8):
        y += res.results[c]["y"].astype(np.float64)
    return y.astype(np.float32).reshape(2, 4096, HID)
